# revision 5
# baseline (speedup 1.0000x reference)
"""Trainium2 Bass kernel for a transformer block: DyT-prenorm attention (RoPE,
causal+mask) + top-2-of-16 MoE with a shared expert.

Strategy (8 NeuronCores, SPMD single program, per-core data):
  * Attention head-parallel: core c computes head c with a transposed-softmax
    layout: scoresT [k, q] per k-chunk, exp with no max-subtraction (scores are
    tiny; masked entries underflow to exactly 0), per-query exp-sums collected
    free via a ones-column appended to v, normalization applied once at the end
    (reciprocal row + rank-1 broadcast matmul).  avT [64, S] is AllGathered
    (bf16) so every core can run the wo projection + residual + dyt locally.
  * A dummy 256-byte AllGather is issued first thing so the runtime's one-time
    collective rendezvous/setup overlaps the load+attention phase instead of
    stalling the real AllGather.
  * PE warm-up matmuls + activation-table preloads run during the initial DMA
    wait so the tensor engine is at full clock when real work starts.
  * Router in bf16 (top-2 selected on logits; gates from exp/sum) - near-tie
    expert flips are bounded by near-equal gates.
  * Experts expert-parallel: 2 dense experts per core in fp8 (DoubleRow mode,
    2x throughput). ek/ev are pre-scaled by 16 host-side to stay in fp8e4's
    normal range; the gelu folds in the 1/16, the gate broadcast carries the
    other 16, and the net x256 is divided out after the ReduceScatter.
    Shared expert sharded over FF (64 cols per core) in bf16, sv pre-scaled
    by 256 so it accumulates into the same PSUM.
  * MoE partials are ReduceScattered in fp8 (halves the payload); the exact
    fp32 residual rows (x rows + wo partials) are added locally after.
Everything is computed transposed (d on partitions, tokens on the free axis);
the host transposes the output back.
"""

import os
import numpy as np
import ml_dtypes

BF = ml_dtypes.bfloat16
F8 = ml_dtypes.float8_e4m3  # TRN fp8e4 (bias 7, max 240) == ml_dtypes e4m3

S = 512      # tokens (B=1)
Dm = 512     # d_model
H = 8        # heads
HD = 64      # head dim
E = 16       # experts
FF = 512     # expert hidden
P = 128
NCORES = 8
DC = Dm // P    # 4 d-model chunks
TCH = S // P    # 4 token chunks
FCH = FF // P   # 4 ff chunks
SHF = FF // NCORES  # shared-expert ff slice per core (64)
DSH = Dm // NCORES  # output row shard per core (64)

WS = 16.0        # fp8 weight pre-scale (ek, ev); gate carries another x16
OSC = 1.0 / (WS * WS)  # net scale folded out after the ReduceScatter

GELU_C = float(2.0 * np.sqrt(2.0 / np.pi))  # sigmoid-form tanh-gelu scale
GELU_A = 0.044715

_PROG_CACHE = {}

LAST_INFO = {}


def _layouts(full_mask):
    """Column layouts of the packed constant arrays (shared host/device)."""
    def lay(blocks):
        off, out = 0, {}
        for name, cols in blocks:
            out[name] = (off, cols)
            off += cols
        return out, off

    atn, atn_c = lay([
        ("wqk", DC * P), ("wv", DC * HD), ("cd", S), ("cs", S),
        ("mask", TCH * (S if full_mask else P)), ("idbf", P), ("ones", P),
    ])
    mid, mid_c = lay([
        ("wo", DC * Dm), ("gw", DC * E), ("sk", DC * SHF), ("sv", Dm),
    ])
    moe, moe_c = lay([
        ("ek", 2 * FCH * 2 * 2 * P), ("ev", DC * 2 * 2 * 2 * P),
    ])
    p32, p32_c = lay([
        ("g1", DC), ("b1", DC), ("g2", DC), ("b2", DC),
        ("gb", TCH * E), ("smsk", 2 * DC), ("idf", P), ("xrows", S),
    ])
    return (atn, atn_c), (mid, mid_c), (moe, moe_c), (p32, p32_c)


def _build_program(a1v: float, a2v: float, full_mask: bool, sim_gelu: bool):
    import concourse.bass as bass
    import concourse.mybir as mybir
    import concourse.tile as tile
    from concourse import bacc

    f32 = mybir.dt.float32
    bf16 = mybir.dt.bfloat16
    f8 = mybir.dt.float8e4
    Alu = mybir.AluOpType
    Act = mybir.ActivationFunctionType
    AX = mybir.AxisListType
    DR = mybir.MatmulPerfMode.DoubleRow
    ts = bass.ts

    (atn_l, atn_c), (mid_l, mid_c), (moe_l, moe_c), (p32_l, p32_c) = \
        _layouts(full_mask)

    nc = bacc.Bacc(
        "TRN2", target_bir_lowering=False, debug=False, num_devices=NCORES
    )

    def inp(name, shape, dt=f32):
        return nc.dram_tensor(name, list(shape), dt, kind="ExternalInput").ap()

    warm_d = inp("warm16", (P, P), bf16)
    xT_d = inp("xT16", (P, DC * S), bf16)
    atn_d = inp("atn16", (P, atn_c), bf16)
    mid_d = inp("mid16", (P, mid_c), bf16)
    moe_d = inp("moe8", (P, moe_c), f8)
    p32_d = inp("p32", (P, p32_c))

    outT_d = nc.dram_tensor("outT", [DSH, S], f32, kind="ExternalOutput").ap()

    with tile.TileContext(nc, num_cores=NCORES) as tc:
        with (
            tc.tile_pool(name="cst", bufs=1) as cst,
            tc.tile_pool(name="tmp", bufs=3) as tmp,
            tc.tile_pool(name="ps", bufs=2, space="PSUM") as psp,
            tc.tile_pool(name="dram", bufs=1, space="DRAM") as drp,
        ):
            # ---------- packed loads (6 DMA dispatches, 2 queues) ----------
            warm16 = cst.tile((P, P), bf16, name="warm16", tag="warm16")
            nc.sync.dma_start(warm16[:], warm_d[:])
            pk32 = cst.tile((P, p32_c), f32, name="pk32", tag="pk32")
            nc.scalar.dma_start(pk32[:], p32_d[:])
            atn16 = cst.tile((P, atn_c), bf16, name="atn16", tag="atn16")
            nc.sync.dma_start(atn16[:], atn_d[:])
            mid16 = cst.tile((P, mid_c), bf16, name="mid16", tag="mid16")
            nc.scalar.dma_start(mid16[:], mid_d[:])
            xTt = cst.tile((P, DC * S), bf16, name="xTt", tag="xTt")
            nc.sync.dma_start(xTt[:], xT_d[:])
            moe8 = cst.tile((P, moe_c), f8, name="moe8", tag="moe8")
            nc.scalar.dma_start(moe8[:], moe_d[:])

            def asl(name, c=0, w=None):  # attention-pack slice
                off, cols = atn_l[name]
                w = cols if w is None else w
                return atn16[:, off + c * w: off + (c + 1) * w]

            def gsl(name, c=0, w=None):  # mid-pack slice
                off, cols = mid_l[name]
                w = cols if w is None else w
                return mid16[:, off + c * w: off + (c + 1) * w]

            def psl(name, c=0, w=None):  # fp32-pack slice
                off, cols = p32_l[name]
                w = cols if w is None else w
                return pk32[:, off + c * w: off + (c + 1) * w]

            def ek8(el, fc, p):  # [128, 2, 128] DoubleRow lhsT (d-pair p)
                off = moe_l["ek"][0] + ((el * FCH + fc) * 2 + p) * 2 * P
                return moe8[:, off: off + 2 * P].rearrange(
                    "p (k m) -> p k m", k=2)

            def ev8(m, el, p):   # [128, 2, 128] DoubleRow lhsT (ff-pair p)
                off = moe_l["ev"][0] + ((m * 2 + el) * 2 + p) * 2 * P
                return moe8[:, off: off + 2 * P].rearrange(
                    "p (k m) -> p k m", k=2)

            idbf = asl("idbf")
            idf = psl("idf")
            ones = asl("ones")

            # ---------- early dummy collective: absorb CC setup ----------
            dum_in = drp.tile((1, P), bf16, name="dum_in")
            dum_out = drp.tile((NCORES, P), bf16, name="dum_out",
                               addr_space="Shared")
            nc.sync.dma_start(dum_in[:], warm_d[0:1, :])
            nc.gpsimd.collective_compute(
                "AllGather", Alu.bypass,
                replica_groups=[list(range(NCORES))],
                ins=[dum_in[:]], outs=[dum_out[:]],
            )

            # ---------- act-table preloads + PE warm-up ----------
            actw = tmp.tile((1, 4), bf16, name="actw", tag="actw", bufs=1)
            nc.scalar.activation(actw[:, 0:1], warm16[0:1, 0:1], Act.Tanh)
            nc.scalar.activation(actw[:, 1:2], warm16[0:1, 0:1], Act.Exp)
            if not sim_gelu:
                nc.scalar.activation(actw[:, 2:3], warm16[0:1, 0:1],
                                     Act.Gelu_apprx_tanh)
            else:
                nc.scalar.activation(actw[:, 2:3], warm16[0:1, 0:1],
                                     Act.Sigmoid)
            warm_ps = psp.tile((P, P), f32, name="warm_ps", tag="avT", bufs=1)
            for _ in range(24):
                nc.tensor.matmul(warm_ps[:], lhsT=warm16[:], rhs=warm16[:],
                                 start=True, stop=True)

            # ---------- phase 1: dyt1 + per-head attention ----------
            hT16 = []
            for c in range(DC):
                th = tmp.tile((P, S), bf16, name="th", tag="t16")
                nc.scalar.activation(th[:], xTt[:, ts(c, S)], Act.Tanh,
                                     scale=float(a1v))
                ht = cst.tile((P, S), bf16, name=f"hT16_{c}", tag=f"hT16_{c}")
                nc.vector.scalar_tensor_tensor(
                    ht[:], th[:], psl("g1", c, 1),
                    psl("b1", c, 1).to_broadcast((P, S)),
                    op0=Alu.mult, op1=Alu.add,
                )
                hT16.append(ht)

            # qkT = [wq*0.125 | wk]^T @ h  -> [128 (q64|k64), S]
            qk_ps = psp.tile((P, S), f32, name="qk_ps", tag="mm")
            for c in range(DC):
                nc.tensor.matmul(
                    qk_ps[:], lhsT=asl("wqk", c, P), rhs=hT16[c][:],
                    start=(c == 0), stop=(c == DC - 1),
                )

            # v (untransposed) with a ones-column at 64: [t-chunk][128, 65]
            v16 = []
            for t in range(TCH):
                v_ps = psp.tile((P, HD), f32, name="v_ps", tag="mm")
                for c in range(DC):
                    nc.tensor.matmul(
                        v_ps[:], lhsT=hT16[c][:, ts(t, P)], rhs=asl("wv", c, HD),
                        start=(c == 0), stop=(c == DC - 1),
                    )
                vt = cst.tile((P, HD + 1), bf16, name=f"v16_{t}", tag=f"v16_{t}")
                nc.vector.tensor_copy(vt[:, 0:HD], v_ps[:])
                nc.vector.tensor_copy(vt[:, HD:HD + 1], ones[:, 0:1])
                v16.append(vt)

            # rope on packed qk
            r1 = tmp.tile((P, S), f32, name="r1", tag="t32")
            nc.vector.tensor_tensor(r1[:], qk_ps[:], asl("cd"), Alu.mult)
            sw = tmp.tile((P, S), f32, name="sw", tag="t32")
            half = HD // 2  # 32
            swap_src = [1, 0, 3, 2]  # 32-row block read for each output block
            cs_ap = asl("cs")
            for b in range(4):
                nc.vector.tensor_tensor(
                    sw[b * half:(b + 1) * half, :],
                    qk_ps[swap_src[b] * half:(swap_src[b] + 1) * half, :],
                    cs_ap[b * half:(b + 1) * half, :],
                    Alu.mult,
                )
            qrot = cst.tile((HD, S), bf16, name="qrot", tag="qrot")
            nc.vector.tensor_tensor(qrot[:], r1[0:HD, :], sw[0:HD, :], Alu.add)
            krot = cst.tile((HD, S), bf16, name="krot", tag="krot")
            nc.vector.tensor_tensor(krot[:], r1[HD:P, :], sw[HD:P, :], Alu.add)

            # scoresT [k, q] per k-chunk; exp with no max subtraction
            mw = S if full_mask else P
            e16 = []
            for j in range(TCH):
                L = S - P * j
                sc_ps = psp.tile((P, S), f32, name="sc_ps", tag="mm")
                nc.tensor.matmul(
                    sc_ps[:, :L], lhsT=krot[:, ts(j, P)], rhs=qrot[:, P * j:S],
                    start=True, stop=True,
                )
                if full_mask:
                    nc.vector.tensor_tensor(
                        sc_ps[:, :L], sc_ps[:, :L], asl("mask", j, mw)[:, :L],
                        Alu.add,
                    )
                else:
                    nc.vector.tensor_tensor(
                        sc_ps[:, 0:P], sc_ps[:, 0:P], asl("mask", j, mw),
                        Alu.add,
                    )
                ej = cst.tile((P, S), bf16, name=f"e16_{j}", tag=f"e16_{j}")
                nc.scalar.activation(ej[:, :L], sc_ps[:, :L], Act.Exp,
                                     scale=1.0)
                e16.append(ej)

            # avT [0:64] + exp-sums row [64] via the v ones-column
            avT_ps = psp.tile((HD + 1, S), f32, name="avT_ps", tag="avT",
                              bufs=1)
            for qi in range(TCH):
                for j in range(qi + 1):
                    nc.tensor.matmul(
                        avT_ps[:, ts(qi, P)], lhsT=v16[j][:],
                        rhs=e16[j][:, ts(qi - j, P)],
                        start=(j == 0), stop=(j == qi),
                    )

            # normalize: reciprocal of sums row, rank-1 broadcast, multiply
            rinv16 = cst.tile((1, S), bf16, name="rinv16", tag="rinv16")
            with nc.allow_low_precision(
                reason="softmax 1/sum in bf16 feeds a bf16 matmul broadcast"
            ):
                nc.vector.reciprocal(rinv16[:], avT_ps[HD:HD + 1, :])
            bc_ps = psp.tile((HD, S), f32, name="bc_ps", tag="lg", bufs=1)
            nc.tensor.matmul(bc_ps[:], lhsT=ones[0:1, 0:HD], rhs=rinv16[:],
                             start=True, stop=True)
            bc16 = tmp.tile((HD, S), bf16, name="bc16", tag="bc16", bufs=1)
            nc.vector.tensor_copy(bc16[:], bc_ps[:])
            ao16 = cst.tile((HD, S), bf16, name="ao16", tag="ao16")
            nc.vector.tensor_tensor(ao16[:], avT_ps[0:HD, :], bc16[:],
                                    Alu.mult)

            # ---------- AllGather attention outputs (heads) ----------
            ag_in = drp.tile((HD, S), bf16, name="ag_in")
            ag_out = drp.tile((H * HD, S), bf16, name="ag_out",
                              addr_space="Shared")
            nc.sync.dma_start(ag_in[:], ao16[:])
            nc.gpsimd.collective_compute(
                "AllGather", Alu.bypass,
                replica_groups=[list(range(NCORES))],
                ins=[ag_in[:]], outs=[ag_out[:]],
            )
            aoT16 = []
            for c in range(DC):
                t = cst.tile((P, S), bf16, name=f"aoT16_{c}", tag=f"aoT16_{c}")
                nc.sync.dma_start(t[:], ag_out[ts(c, P), :])
                aoT16.append(t)

            # ---------- wo projection + residual + dyt2 ----------
            # xsel accumulates the selected 64 rows of (wo partials); the exact
            # fp32 x rows are added at the end (residual stays fp32-exact).
            xsel = cst.tile((DSH, S), f32, name="xsel", tag="xsel")
            h2T16 = []
            h2f8 = [
                cst.tile((P, 2, S), f8, name=f"h2f8_{p}", tag=f"h2f8_{p}")
                for p in range(2)
            ]
            for m in range(DC):
                pw = psp.tile((P, S), f32, name="pw", tag="mm")
                for k in range(DC):
                    nc.tensor.matmul(
                        pw[:], lhsT=gsl("wo", 0)[:, k * Dm + m * P:
                                                 k * Dm + (m + 1) * P],
                        rhs=aoT16[k][:],
                        start=(k == 0), stop=(k == DC - 1),
                    )
                for hh in range(2):
                    j = m * 2 + hh
                    src = pw[hh * DSH:(hh + 1) * DSH, :]
                    if j == 0:
                        nc.vector.tensor_scalar(
                            xsel[:], src, psl("smsk", j, 1)[0:DSH, :], None,
                            op0=Alu.mult,
                        )
                    else:
                        nc.vector.scalar_tensor_tensor(
                            xsel[:], src, psl("smsk", j, 1)[0:DSH, :], xsel[:],
                            op0=Alu.mult, op1=Alu.add,
                        )
                x1b = tmp.tile((P, S), bf16, name="x1b", tag="t16")
                nc.vector.tensor_tensor(x1b[:], pw[:], xTt[:, ts(m, S)],
                                        Alu.add)
                th2 = tmp.tile((P, S), bf16, name="th2", tag="t16")
                nc.scalar.activation(th2[:], x1b[:], Act.Tanh, scale=float(a2v))
                h216 = cst.tile((P, S), bf16, name=f"h2T16_{m}",
                                tag=f"h2T16_{m}")
                nc.vector.scalar_tensor_tensor(
                    h216[:], th2[:], psl("g2", m, 1),
                    psl("b2", m, 1).to_broadcast((P, S)),
                    op0=Alu.mult, op1=Alu.add,
                )
                h2T16.append(h216)
                nc.vector.tensor_copy(h2f8[m // 2][:, m % 2, :], h216[:])
            nc.vector.tensor_tensor(
                xsel[:], xsel[:], psl("xrows")[0:DSH, :], Alu.add)

            # ---------- router (bf16 matmul, fp32 top-2 on logits) ----------
            lg_ps = psp.tile((P, TCH, E), f32, name="lg_ps", tag="lg", bufs=1)
            for t in range(TCH):
                for c in range(DC):
                    nc.tensor.matmul(
                        lg_ps[:, t, :], lhsT=h2T16[c][:, ts(t, P)],
                        rhs=gsl("gw", c, E),
                        start=(c == 0), stop=(c == DC - 1),
                    )
            gb_ap = psl("gb").rearrange("p (t e) -> p t e", e=E)
            lg32 = cst.tile((P, TCH, E), f32, name="lg32", tag="lg32")
            nc.vector.tensor_tensor(lg32[:], lg_ps[:], gb_ap, Alu.add)
            ex32 = cst.tile((P, TCH, E), f32, name="ex32", tag="ex32")
            nc.scalar.activation(ex32[:], lg32[:], Act.Exp, scale=1.0)
            ssum4 = cst.tile((P, TCH), f32, name="ssum4", tag="ssum4")
            nc.vector.reduce_sum(ssum4[:], ex32[:], axis=AX.X)
            rinv4 = cst.tile((P, TCH), f32, name="rinv4", tag="rinv4")
            nc.vector.reciprocal(rinv4[:], ssum4[:])
            m1 = cst.tile((P, TCH), f32, name="m1", tag="m1")
            nc.vector.reduce_max(m1[:], lg32[:], axis=AX.X)
            ge1 = cst.tile((P, TCH, E), f32, name="ge1", tag="ge1")
            nc.vector.tensor_tensor(
                ge1[:], lg32[:], m1[:, :, None].to_broadcast((P, TCH, E)),
                Alu.is_ge,
            )
            msk = cst.tile((P, TCH, E), f32, name="msk", tag="msk")
            nc.vector.scalar_tensor_tensor(
                msk[:], ge1[:], -1e30, lg32[:], op0=Alu.mult, op1=Alu.add
            )
            m2 = cst.tile((P, TCH), f32, name="m2", tag="m2")
            nc.vector.reduce_max(m2[:], msk[:], axis=AX.X)
            ge2 = cst.tile((P, TCH, E), f32, name="ge2", tag="ge2")
            nc.vector.tensor_tensor(
                ge2[:], lg32[:], m2[:, :, None].to_broadcast((P, TCH, E)),
                Alu.is_ge,
            )
            wgt = cst.tile((P, TCH, E), f32, name="wgt", tag="wgt")
            nc.vector.tensor_tensor(wgt[:], ex32[:], ge2[:], Alu.mult)
            wg = cst.tile((P, TCH, E), f32, name="wg", tag="wg")
            nc.vector.tensor_tensor(
                wg[:], wgt[:], rinv4[:, :, None].to_broadcast((P, TCH, E)),
                Alu.mult,
            )

            # transpose the two local experts' gate columns, broadcast across
            # partitions with a rank-1 matmul, scale by 16 (fp8 headroom)
            wrow = [
                cst.tile((1, S), bf16, name=f"wrow{el}", tag=f"wrow{el}")
                for el in range(2)
            ]
            for t in range(TCH):
                for el in range(2):
                    wt_ps = psp.tile((1, P), f32, name="wt_ps",
                                     tag=("lg" if el else "avT"), bufs=1)
                    nc.tensor.transpose(wt_ps[:], wg[:, t, el:el + 1], idf)
                    nc.vector.tensor_copy(wrow[el][:, ts(t, P)], wt_ps[:])
            rep16 = []
            for el in range(2):
                rp_ps = psp.tile((P, S), f32, name="rp_ps", tag="mm")
                nc.tensor.matmul(
                    rp_ps[:], lhsT=ones[0:1, :], rhs=wrow[el][:],
                    start=True, stop=True,
                )
                rp = cst.tile((P, S), bf16, name=f"rep16_{el}",
                              tag=f"rep16_{el}")
                nc.vector.tensor_scalar(rp[:], rp_ps[:], float(WS), None,
                                        op0=Alu.mult)
                rep16.append(rp)

            # ---------- expert ups (fp8 DoubleRow) + gelu + gate ----------
            def gelu_gated(up_ps, dst, rep):
                """dst (fp8 slice) = gelu(up_ps/16) * rep."""
                g0 = tmp.tile((P, S), bf16, name="g0", tag="g16", bufs=4)
                if not sim_gelu:
                    nc.scalar.activation(g0[:], up_ps[:], Act.Gelu_apprx_tanh,
                                         scale=1.0 / WS)
                else:
                    u16 = tmp.tile((P, S), bf16, name="u16", tag="u16", bufs=2)
                    nc.vector.tensor_scalar(u16[:], up_ps[:], 1.0 / WS, None,
                                            op0=Alu.mult)
                    x2 = tmp.tile((P, S), bf16, name="x2", tag="x2", bufs=2)
                    nc.vector.tensor_tensor(x2[:], u16[:], u16[:], Alu.mult)
                    t1 = tmp.tile((P, S), bf16, name="t1", tag="x2", bufs=2)
                    nc.vector.tensor_scalar(t1[:], x2[:], GELU_A, 1.0,
                                            op0=Alu.mult, op1=Alu.add)
                    mm_ = tmp.tile((P, S), bf16, name="mm_", tag="x2", bufs=2)
                    nc.vector.tensor_tensor(mm_[:], u16[:], t1[:], Alu.mult)
                    sg = tmp.tile((P, S), bf16, name="sg", tag="x2", bufs=2)
                    nc.scalar.activation(sg[:], mm_[:], Act.Sigmoid,
                                         scale=GELU_C)
                    nc.vector.tensor_tensor(g0[:], u16[:], sg[:], Alu.mult)
                nc.vector.tensor_tensor(dst, g0[:], rep[:], Alu.mult)

            g0f8 = [
                [
                    cst.tile((P, 2, S), f8, name=f"g0f8_{el}_{p}",
                             tag=f"g0f8_{el}_{p}")
                    for p in range(2)
                ]
                for el in range(2)
            ]
            for el in range(2):
                for fc in range(FCH):
                    up_ps = psp.tile((P, S), f32, name="up_ps", tag="mm")
                    for p in range(2):
                        nc.tensor.matmul(
                            up_ps[:], lhsT=ek8(el, fc, p), rhs=h2f8[p][:],
                            start=(p == 0), stop=(p == 1), perf_mode=DR,
                        )
                    gelu_gated(up_ps, g0f8[el][fc // 2][:, fc % 2, :],
                               rep16[el])

            # shared expert (bf16; sv pre-scaled x256 to match expert scale)
            su_ps = psp.tile((SHF, S), f32, name="su_ps", tag="mm")
            for c in range(DC):
                nc.tensor.matmul(
                    su_ps[:], lhsT=gsl("sk", c, SHF), rhs=h2T16[c][:],
                    start=(c == 0), stop=(c == DC - 1),
                )
            gs16 = cst.tile((SHF, S), bf16, name="gs16", tag="gs16")
            if not sim_gelu:
                nc.scalar.activation(gs16[:], su_ps[:], Act.Gelu_apprx_tanh)
            else:
                su16 = tmp.tile((SHF, S), bf16, name="su16", tag="u16", bufs=2)
                nc.vector.tensor_copy(su16[:], su_ps[:])
                sx2 = tmp.tile((SHF, S), bf16, name="sx2", tag="x2", bufs=2)
                nc.vector.tensor_tensor(sx2[:], su16[:], su16[:], Alu.mult)
                st1 = tmp.tile((SHF, S), bf16, name="st1", tag="x2", bufs=2)
                nc.vector.tensor_scalar(st1[:], sx2[:], GELU_A, 1.0,
                                        op0=Alu.mult, op1=Alu.add)
                smm = tmp.tile((SHF, S), bf16, name="smm", tag="x2", bufs=2)
                nc.vector.tensor_tensor(smm[:], su16[:], st1[:], Alu.mult)
                ssg = tmp.tile((SHF, S), bf16, name="ssg", tag="x2", bufs=2)
                nc.scalar.activation(ssg[:], smm[:], Act.Sigmoid, scale=GELU_C)
                nc.vector.tensor_tensor(gs16[:], su16[:], ssg[:], Alu.mult)

            # ---------- down-projections (fp8 DoubleRow) ----------
            rs_in = drp.tile((Dm, S), f8, name="rs_in")
            for m in range(DC):
                moe_ps = psp.tile((P, S), f32, name=f"moe_ps{m}", tag="moe",
                                  bufs=4)
                first = True
                for el in range(2):
                    for p in range(2):
                        nc.tensor.matmul(
                            moe_ps[:], lhsT=ev8(m, el, p), rhs=g0f8[el][p][:],
                            start=first, stop=False, perf_mode=DR,
                        )
                        first = False
                nc.tensor.matmul(
                    moe_ps[:], lhsT=gsl("sv", m, P)[0:SHF, :], rhs=gs16[:],
                    start=False, stop=True,
                )
                fin = tmp.tile((P, S), f8, name="fin", tag="fin", bufs=2)
                nc.vector.tensor_copy(fin[:], moe_ps[:])
                nc.sync.dma_start(rs_in[ts(m, P), :], fin[:])

            # ---------- fp8 ReduceScatter of MoE + exact local residual ----
            rs_out = drp.tile((DSH, S), f8, name="rs_out")
            nc.gpsimd.collective_compute(
                "ReduceScatter", Alu.add,
                replica_groups=[list(range(NCORES))],
                ins=[rs_in[:]], outs=[rs_out[:]],
            )
            rs_sb = cst.tile((DSH, S), f8, name="rs_sb", tag="rs_sb")
            nc.sync.dma_start(rs_sb[:], rs_out[:])
            out32 = cst.tile((DSH, S), f32, name="out32", tag="out32")
            nc.vector.scalar_tensor_tensor(
                out32[:], rs_sb[:], float(OSC), xsel[:],
                op0=Alu.mult, op1=Alu.add,
            )
            nc.sync.dma_start(outT_d[:], out32[:])

    nc.compile()
    return nc


def _prep_inputs(inputs):
    """Host-side sharding/layout prep. Returns (in_maps, a1, a2, full_mask)."""
    x = np.asarray(inputs["x"], np.float32)            # [1,S,D]
    attn_mask = np.asarray(inputs["attn_mask"])        # [1,S]
    wq = np.asarray(inputs["wq"], np.float32)
    wk = np.asarray(inputs["wk"], np.float32)
    wv = np.asarray(inputs["wv"], np.float32)
    wo = np.asarray(inputs["wo"], np.float32)
    a1 = float(np.asarray(inputs["a1"]).reshape(-1)[0])
    g1 = np.asarray(inputs["g1"], np.float32).reshape(Dm)
    b1 = np.asarray(inputs["b1"], np.float32).reshape(Dm)
    a2 = float(np.asarray(inputs["a2"]).reshape(-1)[0])
    g2 = np.asarray(inputs["g2"], np.float32).reshape(Dm)
    b2 = np.asarray(inputs["b2"], np.float32).reshape(Dm)
    gate_w = np.asarray(inputs["gate_w"], np.float32)  # [D,E]
    gate_b = np.asarray(inputs["gate_b"], np.float32).reshape(E)
    ek = np.asarray(inputs["ek"], np.float32)          # [E,D,FF]
    ev = np.asarray(inputs["ev"], np.float32)          # [E,FF,D]
    sk = np.asarray(inputs["sk"], np.float32)          # [1,D,FF]
    sv = np.asarray(inputs["sv"], np.float32)          # [1,FF,D]

    xT = np.ascontiguousarray(x[0].T)                  # [D,S]
    # chunk-major pack: [128, 4*512]
    xTp = np.concatenate([xT[i * P:(i + 1) * P, :] for i in range(DC)], axis=1)

    # rope tables (transposed layout: [freq, pos])
    pos = np.arange(S, dtype=np.float32)
    half = HD // 2
    inv = 1.0 / (10000.0 ** (np.arange(half, dtype=np.float32) / half))
    ang = pos[:, None] * inv[None, :]                  # [S, half]
    cosT = np.cos(ang).T.astype(np.float32)            # [32,S]
    sinT = np.sin(ang).T.astype(np.float32)
    cd = np.concatenate([cosT, cosT, cosT, cosT], 0)
    cs = np.concatenate([-sinT, sinT, -sinT, sinT], 0)

    # additive attention mask, exactly as the reference builds it, but stored
    # TRANSPOSED ([k, q]) for the scoresT layout.
    causal = np.tril(np.ones((S, S), np.float32))
    am = attn_mask.astype(np.float32)[0]               # [S]
    cm = causal * am[None, :]
    cm[np.arange(S), np.arange(S)] = 1.0
    addmask = -(1.0 - cm) * 1e9                        # [S,S] ([q, k])
    addmaskT = np.ascontiguousarray(addmask.T)         # [k, q]
    offdiag_needed = any(
        np.any(addmask[i * P:(i + 1) * P, : i * P] != 0.0)
        for i in range(1, TCH)
    )
    full_mask = bool(offdiag_needed)
    if full_mask:
        # block j: [128 k-rows, S q-cols], valid region [:, :S-128j]
        mblocks = []
        for j in range(TCH):
            blk = np.zeros((P, S), np.float32)
            blk[:, :S - P * j] = addmaskT[j * P:(j + 1) * P, P * j:]
            mblocks.append(blk)
    else:
        mblocks = [addmaskT[i * P:(i + 1) * P, i * P:(i + 1) * P]
                   for i in range(TCH)]

    (atn_l, atn_c), (mid_l, mid_c), (moe_l, moe_c), (p32_l, p32_c) = \
        _layouts(full_mask)

    def pack(layout, total, blocks, dtype):
        arr = np.zeros((P, total), dtype)
        for name, data in blocks.items():
            off, cols = layout[name]
            data = np.asarray(data, np.float32)
            assert data.shape[1] == cols, (name, data.shape, cols)
            arr[:data.shape[0], off:off + cols] = data.astype(dtype)
        return arr

    def cat(chunks):
        return np.concatenate(chunks, axis=1)

    wo_pk = cat([wo[i * P:(i + 1) * P, :] for i in range(DC)])
    id128 = np.eye(P, dtype=np.float32)

    common32 = {
        "g1": np.stack([g1[i * P:(i + 1) * P] for i in range(DC)], 1),
        "b1": np.stack([b1[i * P:(i + 1) * P] for i in range(DC)], 1),
        "g2": np.stack([g2[i * P:(i + 1) * P] for i in range(DC)], 1),
        "b2": np.stack([b2[i * P:(i + 1) * P] for i in range(DC)], 1),
        "idf": id128,
    }

    warm = np.eye(P, dtype=BF)

    in_maps = []
    for c in range(NCORES):
        hsl = slice(c * HD, (c + 1) * HD)
        wqk = np.concatenate([wq[:, hsl] * 0.125, wk[:, hsl]], axis=1)
        wqk_pk = cat([wqk[i * P:(i + 1) * P, :] for i in range(DC)])
        wv_pk = cat([wv[i * P:(i + 1) * P, hsl] for i in range(DC)])
        atn_pack = pack(atn_l, atn_c, {
            "wqk": wqk_pk, "wv": wv_pk, "cd": cd, "cs": cs,
            "mask": cat(mblocks), "idbf": id128,
            "ones": np.ones((P, P), np.float32),
        }, BF)

        perm = [2 * c, 2 * c + 1] + [e for e in range(E)
                                     if e not in (2 * c, 2 * c + 1)]
        gwp = gate_w[:, perm]
        gbp = gate_b[perm]
        mid_pack = pack(mid_l, mid_c, {
            "wo": wo_pk,
            "gw": cat([gwp[i * P:(i + 1) * P, :] for i in range(DC)]),
            "sk": cat([sk[0][i * P:(i + 1) * P, c * SHF:(c + 1) * SHF]
                       for i in range(DC)]),
            "sv": sv[0][c * SHF:(c + 1) * SHF, :] * (WS * WS),
        }, BF)

        p32_pack = pack(p32_l, p32_c, dict(
            common32,
            gb=np.tile(gbp, (P, TCH)),
            smsk=np.tile((np.arange(2 * DC) == c).astype(np.float32), (P, 1)),
            xrows=xT[c * DSH:(c + 1) * DSH, :],
        ), np.float32)

        # fp8 DoubleRow packs (pre-scaled by WS)
        ekb = []
        for el in range(2):
            eke = ek[2 * c + el] * WS                  # [D, FF]
            for fc in range(FCH):
                for p in range(2):
                    for k in range(2):
                        d0 = (2 * p + k) * P
                        ekb.append(eke[d0:d0 + P, fc * P:(fc + 1) * P])
        evb = []
        for m in range(DC):
            for el in range(2):
                eve = ev[2 * c + el] * WS              # [FF, D]
                for p in range(2):
                    for k in range(2):
                        f0 = (2 * p + k) * P
                        evb.append(eve[f0:f0 + P, m * P:(m + 1) * P])
        moe_pack = pack(moe_l, moe_c, {
            "ek": cat(ekb), "ev": cat(evb),
        }, F8)

        in_maps.append(dict(
            warm16=warm, xT16=xTp.astype(BF),
            atn16=atn_pack, mid16=mid_pack, moe8=moe_pack, p32=p32_pack,
        ))
    return in_maps, a1, a2, full_mask


def kernel(**inputs):
    from concourse import bass_utils

    sim = bool(os.environ.get("BASSK_SIM"))
    sim_gelu = sim or bool(os.environ.get("BASSK_COMPOSED_GELU"))
    in_maps, a1v, a2v, full_mask = _prep_inputs(inputs)
    key = (a1v, a2v, full_mask, sim_gelu)
    if key not in _PROG_CACHE:
        _PROG_CACHE[key] = _build_program(a1v, a2v, full_mask, sim_gelu)
    nc = _PROG_CACHE[key]

    if sim:
        from concourse.bass_interp import MultiCoreSim

        simu = MultiCoreSim(nc, num_cores=NCORES)
        for c in range(NCORES):
            for k, v in in_maps[c].items():
                simu.cores[c].tensor(k)[:] = v
        simu.simulate(check_with_hw=False)
        shards = [np.array(simu.cores[c].tensor("outT")) for c in range(NCORES)]
    else:
        trace = bool(os.environ.get("BASSK_TRACE"))
        res = bass_utils.run_bass_kernel_spmd(
            nc, in_maps, core_ids=list(range(NCORES)), trace=trace
        )
        LAST_INFO["exec_time_ns"] = res.exec_time_ns
        LAST_INFO["profile_json"] = res.profile_json
        shards = [np.asarray(res.results[c]["outT"]) for c in range(NCORES)]

    outT = np.concatenate(shards, axis=0)              # [D,S]
    return np.ascontiguousarray(outT.T).reshape(1, S, Dm).astype(np.float32)


# revision 7
# speedup vs baseline: 1.1397x; 1.1397x over previous
"""Trainium2 Bass kernel for a transformer block: DyT-prenorm attention (RoPE,
causal+mask) + top-2-of-16 MoE with a shared expert.

Strategy (8 NeuronCores, SPMD single program, per-core data):
  * Attention head-parallel: core c computes head c with a transposed-softmax
    layout: scoresT [k, q] per k-chunk, exp with no max-subtraction (scores are
    tiny; masked entries underflow to exactly 0).  Per-query exp-sums are
    accumulated into a [128, 4] PSUM tile with tiny N=1 matmuls (so the
    reciprocal runs on 128 DVE lanes instead of one), transposed back to a row
    and broadcast with 4 rank-1 matmuls.  avT [64, S] is AllGathered (bf16).
  * Loads are ordered so only attention-critical tensors stream first; the wo
    and expert weight packs are enqueued behind the AllGather input DMA so
    they never compete with the attention-phase DMAs.
  * The residual rows are computed exactly: xsel = wo[:, own]^T @ aoT (4 extra
    matmuls on the idle tensor engine) + fp32 x rows, added after the
    ReduceScatter.
  * Router in bf16 (top-2 on logits; gates from exp/sum).  Expert up
    projections are emitted BEFORE the gate machinery so the tensor engine
    never waits on the DVE top-2 chain; gates multiply the gelu outputs right
    before the down projections (reading the gate broadcast from PSUM).
  * Experts expert-parallel: 2 dense experts per core in fp8 (DoubleRow mode).
    ek/ev are pre-scaled by 16 host-side to stay in fp8e4's normal range; the
    gelu folds in the 1/16, the gate broadcast carries the other 16, and the
    net x256 is divided out after the ReduceScatter.  Shared expert sharded
    over FF (64 cols per core) in bf16, sv pre-scaled by 256.
  * MoE partials are ReduceScattered in fp8.
  * PE warm-up matmuls + Tanh/Exp activation-table preloads run during the
    initial DMA wait; a dummy Gelu right after dyt2 prefetches its table
    behind the expert up matmuls.
Everything is computed transposed (d on partitions, tokens on the free axis);
the host transposes the output back.
"""

import os
import numpy as np
import ml_dtypes

BF = ml_dtypes.bfloat16
F8 = ml_dtypes.float8_e4m3  # TRN fp8e4 (bias 7, max 240) == ml_dtypes e4m3

S = 512      # tokens (B=1)
Dm = 512     # d_model
H = 8        # heads
HD = 64      # head dim
E = 16       # experts
FF = 512     # expert hidden
P = 128
NCORES = 8
DC = Dm // P    # 4 d-model chunks
TCH = S // P    # 4 token chunks
FCH = FF // P   # 4 ff chunks
SHF = FF // NCORES  # shared-expert ff slice per core (64)
DSH = Dm // NCORES  # output row shard per core (64)

WS = 16.0        # fp8 weight pre-scale (ek, ev); gate carries another x16
OSC = 1.0 / (WS * WS)  # net scale folded out after the ReduceScatter

GELU_C = float(2.0 * np.sqrt(2.0 / np.pi))  # sigmoid-form tanh-gelu scale
GELU_A = 0.044715

_PROG_CACHE = {}

LAST_INFO = {}


def _layouts(full_mask):
    """Column layouts of the packed constant arrays (shared host/device)."""
    def lay(blocks):
        off, out = 0, {}
        for name, cols in blocks:
            out[name] = (off, cols)
            off += cols
        return out, off

    atn, atn_c = lay([
        ("wqk", DC * P), ("wv", DC * HD), ("cd", S), ("cs", S),
        ("mask", TCH * (S if full_mask else P)), ("idbf", P), ("ones", P),
        ("sel4", TCH * HD),
    ])
    mid, mid_c = lay([
        ("wo", DC * Dm), ("wosel", DC * DSH), ("gw", DC * E),
        ("sk", DC * SHF), ("sv", Dm),
    ])
    moe, moe_c = lay([
        ("ek", 2 * FCH * 2 * 2 * P), ("ev", DC * 2 * 2 * 2 * P),
    ])
    p32, p32_c = lay([
        ("g1", DC), ("b1", DC), ("g2", DC), ("b2", DC),
        ("gb", TCH * E), ("idf", P), ("xrows", S),
    ])
    return (atn, atn_c), (mid, mid_c), (moe, moe_c), (p32, p32_c)


def _build_program(a1v: float, a2v: float, full_mask: bool, sim_gelu: bool):
    import concourse.bass as bass
    import concourse.mybir as mybir
    import concourse.tile as tile
    from concourse import bacc

    f32 = mybir.dt.float32
    bf16 = mybir.dt.bfloat16
    f8 = mybir.dt.float8e4
    Alu = mybir.AluOpType
    Act = mybir.ActivationFunctionType
    AX = mybir.AxisListType
    DR = mybir.MatmulPerfMode.DoubleRow
    ts = bass.ts

    (atn_l, atn_c), (mid_l, mid_c), (moe_l, moe_c), (p32_l, p32_c) = \
        _layouts(full_mask)

    nc = bacc.Bacc(
        "TRN2", target_bir_lowering=False, debug=False, num_devices=NCORES
    )

    def inp(name, shape, dt=f32):
        return nc.dram_tensor(name, list(shape), dt, kind="ExternalInput").ap()

    warm_d = inp("warm16", (P, P), bf16)
    xT_d = inp("xT16", (P, DC * S), bf16)
    atn_d = inp("atn16", (P, atn_c), bf16)
    mid_d = inp("mid16", (P, mid_c), bf16)
    moe_d = inp("moe8", (P, moe_c), f8)
    p32_d = inp("p32", (P, p32_c))

    outT_d = nc.dram_tensor("outT", [DSH, S], f32, kind="ExternalOutput").ap()

    with tile.TileContext(nc, num_cores=NCORES) as tc:
        with (
            tc.tile_pool(name="cst", bufs=1) as cst,
            tc.tile_pool(name="tmp", bufs=3) as tmp,
            tc.tile_pool(name="ps", bufs=2, space="PSUM") as psp,
            tc.tile_pool(name="dram", bufs=1, space="DRAM") as drp,
        ):
            # ---------- attention-critical loads only (sync q + pk32) ------
            warm16 = cst.tile((P, P), bf16, name="warm16", tag="warm16")
            nc.sync.dma_start(warm16[:], warm_d[:])
            pk32 = cst.tile((P, p32_c), f32, name="pk32", tag="pk32")
            nc.scalar.dma_start(pk32[:], p32_d[:])
            atn16 = cst.tile((P, atn_c), bf16, name="atn16", tag="atn16")
            nc.sync.dma_start(atn16[:], atn_d[:])
            xTt = cst.tile((P, DC * S), bf16, name="xTt", tag="xTt")
            nc.sync.dma_start(xTt[:, 0: 2 * S], xT_d[:, 0: 2 * S])
            nc.sync.dma_start(xTt[:, 2 * S: 4 * S], xT_d[:, 2 * S: 4 * S])

            def asl(name, c=0, w=None):  # attention-pack slice
                off, cols = atn_l[name]
                w = cols if w is None else w
                return atn16[:, off + c * w: off + (c + 1) * w]

            def gsl(name, c=0, w=None):  # mid-pack slice
                off, cols = mid_l[name]
                w = cols if w is None else w
                return mid16[:, off + c * w: off + (c + 1) * w]

            def psl(name, c=0, w=None):  # fp32-pack slice
                off, cols = p32_l[name]
                w = cols if w is None else w
                return pk32[:, off + c * w: off + (c + 1) * w]

            idbf = asl("idbf")
            idf = psl("idf")
            ones = asl("ones")

            # ---------- act-table preloads + PE warm-up ----------
            actw = tmp.tile((1, 4), bf16, name="actw", tag="actw", bufs=1)
            nc.scalar.activation(actw[:, 0:1], warm16[0:1, 0:1], Act.Tanh)
            nc.scalar.activation(actw[:, 1:2], warm16[0:1, 0:1], Act.Exp)
            warm_ps = psp.tile((P, P), f32, name="warm_ps", tag="avT", bufs=1)
            for _ in range(24):
                nc.tensor.matmul(warm_ps[:], lhsT=warm16[:], rhs=warm16[:],
                                 start=True, stop=True)

            # ---------- phase 1: dyt1 + per-head attention ----------
            hT16 = []
            for c in range(DC):
                th = tmp.tile((P, S), bf16, name="th", tag="t16")
                nc.scalar.activation(th[:], xTt[:, ts(c, S)], Act.Tanh,
                                     scale=float(a1v))
                ht = cst.tile((P, S), bf16, name=f"hT16_{c}", tag=f"hT16_{c}")
                nc.vector.scalar_tensor_tensor(
                    ht[:], th[:], psl("g1", c, 1),
                    psl("b1", c, 1).to_broadcast((P, S)),
                    op0=Alu.mult, op1=Alu.add,
                )
                hT16.append(ht)

            # qkT = [wq*0.125 | wk]^T @ h  -> [128 (q64|k64), S]
            qk_ps = psp.tile((P, S), f32, name="qk_ps", tag="mm")
            for c in range(DC):
                nc.tensor.matmul(
                    qk_ps[:], lhsT=asl("wqk", c, P), rhs=hT16[c][:],
                    start=(c == 0), stop=(c == DC - 1),
                )

            # v (untransposed): [t-chunk][128, 64]
            v16 = []
            for t in range(TCH):
                v_ps = psp.tile((P, HD), f32, name="v_ps", tag="mm")
                for c in range(DC):
                    nc.tensor.matmul(
                        v_ps[:], lhsT=hT16[c][:, ts(t, P)], rhs=asl("wv", c, HD),
                        start=(c == 0), stop=(c == DC - 1),
                    )
                vt = cst.tile((P, HD), bf16, name=f"v16_{t}", tag=f"v16_{t}")
                nc.vector.tensor_copy(vt[:], v_ps[:])
                v16.append(vt)

            # rope on packed qk (bf16 intermediates - 2x DVE rate)
            r1 = tmp.tile((P, S), bf16, name="r1", tag="t16")
            nc.vector.tensor_tensor(r1[:], qk_ps[:], asl("cd"), Alu.mult)
            sw = tmp.tile((P, S), bf16, name="sw", tag="t16")
            half = HD // 2  # 32
            swap_src = [1, 0, 3, 2]  # 32-row block read for each output block
            cs_ap = asl("cs")
            for b in range(4):
                nc.vector.tensor_tensor(
                    sw[b * half:(b + 1) * half, :],
                    qk_ps[swap_src[b] * half:(swap_src[b] + 1) * half, :],
                    cs_ap[b * half:(b + 1) * half, :],
                    Alu.mult,
                )
            qrot = cst.tile((HD, S), bf16, name="qrot", tag="qrot")
            nc.vector.tensor_tensor(qrot[:], r1[0:HD, :], sw[0:HD, :], Alu.add)
            krot = cst.tile((HD, S), bf16, name="krot", tag="krot")
            nc.vector.tensor_tensor(krot[:], r1[HD:P, :], sw[HD:P, :], Alu.add)

            # scoresT [k, q] per k-chunk; exp with no max subtraction
            mw = S if full_mask else P
            e16 = []
            for j in range(TCH):
                L = S - P * j
                sc_ps = psp.tile((P, S), f32, name="sc_ps", tag="mm")
                nc.tensor.matmul(
                    sc_ps[:, :L], lhsT=krot[:, ts(j, P)], rhs=qrot[:, P * j:S],
                    start=True, stop=True,
                )
                if full_mask:
                    nc.vector.tensor_tensor(
                        sc_ps[:, :L], sc_ps[:, :L], asl("mask", j, mw)[:, :L],
                        Alu.add,
                    )
                else:
                    nc.vector.tensor_tensor(
                        sc_ps[:, 0:P], sc_ps[:, 0:P], asl("mask", j, mw),
                        Alu.add,
                    )
                ej = cst.tile((P, S), bf16, name=f"e16_{j}", tag=f"e16_{j}")
                nc.scalar.activation(ej[:, :L], sc_ps[:, :L], Act.Exp,
                                     scale=1.0)
                e16.append(ej)

            # avT accumulation; exp-sums into [128, 4] via tiny N=1 matmuls
            avT_ps = psp.tile((HD, S), f32, name="avT_ps", tag="avT", bufs=1)
            sums_ps = psp.tile((P, TCH), f32, name="sums_ps", tag="lg",
                               bufs=1)
            for qi in range(TCH):
                for j in range(qi + 1):
                    nc.tensor.matmul(
                        avT_ps[:, ts(qi, P)], lhsT=v16[j][:],
                        rhs=e16[j][:, ts(qi - j, P)],
                        start=(j == 0), stop=(j == qi),
                    )
                for j in range(qi + 1):
                    nc.tensor.matmul(
                        sums_ps[:, qi:qi + 1], lhsT=e16[j][:, ts(qi - j, P)],
                        rhs=ones[:, 0:1],
                        start=(j == 0), stop=(j == qi),
                    )

            # normalize: 128-lane reciprocal, transpose to a row, rank-1 bcast
            rinv128 = cst.tile((P, TCH), bf16, name="rinv128", tag="rinv128")
            with nc.allow_low_precision(
                reason="softmax 1/sum in bf16 feeds a bf16 matmul broadcast"
            ):
                nc.vector.reciprocal(rinv128[:], sums_ps[:])
            rT_ps = psp.tile((TCH, P), bf16, name="rT_ps", tag="lg", bufs=1)
            nc.tensor.transpose(rT_ps[:], rinv128[:], idbf)
            rT16 = cst.tile((TCH, P), bf16, name="rT16", tag="rT16")
            nc.vector.tensor_copy(rT16[:], rT_ps[:])
            bc_ps = psp.tile((HD, S), f32, name="bc_ps", tag="lg", bufs=1)
            for i in range(TCH):
                nc.tensor.matmul(bc_ps[:, ts(i, P)],
                                 lhsT=asl("sel4", i, HD)[0:TCH, :],
                                 rhs=rT16[:], start=True, stop=True)
            bc16 = tmp.tile((HD, S), bf16, name="bc16", tag="bc16", bufs=1)
            nc.vector.tensor_copy(bc16[:], bc_ps[:])
            ao16 = cst.tile((HD, S), bf16, name="ao16", tag="ao16")
            nc.vector.tensor_tensor(ao16[:], avT_ps[:], bc16[:], Alu.mult)

            # ---------- AllGather attention outputs (heads) ----------
            # The heavy MoE-phase weight packs load behind ag_in on the same
            # queue: they never compete with attention loads, and finish
            # during the AllGather wait.
            ag_in = drp.tile((HD, S), bf16, name="ag_in")
            ag_out = drp.tile((H * HD, S), bf16, name="ag_out",
                              addr_space="Shared")
            nc.sync.dma_start(ag_in[:], ao16[:])
            mid16 = cst.tile((P, mid_c), bf16, name="mid16", tag="mid16")
            nc.sync.dma_start(mid16[:], mid_d[:])
            moe8 = cst.tile((P, moe_c), f8, name="moe8", tag="moe8")
            nc.sync.dma_start(moe8[:], moe_d[:])

            def ek8(el, fc, p):  # [128, 2, 128] DoubleRow lhsT (d-pair p)
                off = moe_l["ek"][0] + ((el * FCH + fc) * 2 + p) * 2 * P
                return moe8[:, off: off + 2 * P].rearrange(
                    "p (k m) -> p k m", k=2)

            def ev8(m, el, p):   # [128, 2, 128] DoubleRow lhsT (ff-pair p)
                off = moe_l["ev"][0] + ((m * 2 + el) * 2 + p) * 2 * P
                return moe8[:, off: off + 2 * P].rearrange(
                    "p (k m) -> p k m", k=2)

            nc.gpsimd.collective_compute(
                "AllGather", Alu.bypass,
                replica_groups=[list(range(NCORES))],
                ins=[ag_in[:]], outs=[ag_out[:]],
            )
            aoT16 = []
            for c in range(DC):
                t = cst.tile((P, S), bf16, name=f"aoT16_{c}", tag=f"aoT16_{c}")
                nc.sync.dma_start(t[:], ag_out[ts(c, P), :])
                aoT16.append(t)

            # ---------- wo projection + residual + dyt2 ----------
            h2T16 = []
            h2f8 = [
                cst.tile((P, 2, S), f8, name=f"h2f8_{p}", tag=f"h2f8_{p}")
                for p in range(2)
            ]
            for m in range(DC):
                pw = psp.tile((P, S), f32, name="pw", tag="mm")
                for k in range(DC):
                    nc.tensor.matmul(
                        pw[:], lhsT=gsl("wo", 0)[:, k * Dm + m * P:
                                                 k * Dm + (m + 1) * P],
                        rhs=aoT16[k][:],
                        start=(k == 0), stop=(k == DC - 1),
                    )
                x1b = tmp.tile((P, S), bf16, name="x1b", tag="t16")
                nc.vector.tensor_tensor(x1b[:], pw[:], xTt[:, ts(m, S)],
                                        Alu.add)
                th2 = tmp.tile((P, S), bf16, name="th2", tag="t16")
                nc.scalar.activation(th2[:], x1b[:], Act.Tanh, scale=float(a2v))
                h216 = cst.tile((P, S), bf16, name=f"h2T16_{m}",
                                tag=f"h2T16_{m}")
                nc.vector.scalar_tensor_tensor(
                    h216[:], th2[:], psl("g2", m, 1),
                    psl("b2", m, 1).to_broadcast((P, S)),
                    op0=Alu.mult, op1=Alu.add,
                )
                h2T16.append(h216)
                nc.vector.tensor_copy(h2f8[m // 2][:, m % 2, :], h216[:])

            # exact residual rows: wo[:, own cols]^T @ aoT + fp32 x rows
            xsel_ps = psp.tile((DSH, S), f32, name="xsel_ps", tag="lg",
                               bufs=1)
            for k in range(DC):
                nc.tensor.matmul(
                    xsel_ps[:], lhsT=gsl("wosel", k, DSH), rhs=aoT16[k][:],
                    start=(k == 0), stop=(k == DC - 1),
                )
            xsel = cst.tile((DSH, S), f32, name="xsel", tag="xsel")
            nc.vector.tensor_tensor(
                xsel[:], xsel_ps[:], psl("xrows")[0:DSH, :], Alu.add)

            # Gelu table prefetch: loads during the expert up matmuls
            if not sim_gelu:
                nc.scalar.activation(actw[:, 2:3], warm16[0:1, 0:1],
                                     Act.Gelu_apprx_tanh)
            else:
                nc.scalar.activation(actw[:, 2:3], warm16[0:1, 0:1],
                                     Act.Sigmoid)

            # ---------- expert ups (fp8 DoubleRow) + gelu (ungated) --------
            def gelu_ungated(up_ps, g0):
                """g0 (bf16) = gelu(up_ps/16)."""
                if not sim_gelu:
                    nc.scalar.activation(g0[:], up_ps[:], Act.Gelu_apprx_tanh,
                                         scale=1.0 / WS)
                    return
                u16 = tmp.tile((P, S), bf16, name="u16", tag="u16", bufs=2)
                nc.vector.tensor_scalar(u16[:], up_ps[:], 1.0 / WS, None,
                                        op0=Alu.mult)
                x2 = tmp.tile((P, S), bf16, name="x2", tag="x2", bufs=2)
                nc.vector.tensor_tensor(x2[:], u16[:], u16[:], Alu.mult)
                t1 = tmp.tile((P, S), bf16, name="t1", tag="x2", bufs=2)
                nc.vector.tensor_scalar(t1[:], x2[:], GELU_A, 1.0,
                                        op0=Alu.mult, op1=Alu.add)
                mm_ = tmp.tile((P, S), bf16, name="mm_", tag="x2", bufs=2)
                nc.vector.tensor_tensor(mm_[:], u16[:], t1[:], Alu.mult)
                sg = tmp.tile((P, S), bf16, name="sg", tag="x2", bufs=2)
                nc.scalar.activation(sg[:], mm_[:], Act.Sigmoid, scale=GELU_C)
                nc.vector.tensor_tensor(g0[:], u16[:], sg[:], Alu.mult)

            g0b = [[None] * FCH for _ in range(2)]
            for el in range(2):
                for fc in range(FCH):
                    up_ps = psp.tile((P, S), f32, name="up_ps", tag="mm")
                    for p in range(2):
                        nc.tensor.matmul(
                            up_ps[:], lhsT=ek8(el, fc, p), rhs=h2f8[p][:],
                            start=(p == 0), stop=(p == 1), perf_mode=DR,
                        )
                    g0 = cst.tile((P, S), bf16, name=f"g0_{el}_{fc}",
                                  tag=f"g0_{el}_{fc}")
                    gelu_ungated(up_ps, g0)
                    g0b[el][fc] = g0

            # shared expert up (bf16; overlaps the router/gate chain)
            su_ps = psp.tile((SHF, S), f32, name="su_ps", tag="mm")
            for c in range(DC):
                nc.tensor.matmul(
                    su_ps[:], lhsT=gsl("sk", c, SHF), rhs=h2T16[c][:],
                    start=(c == 0), stop=(c == DC - 1),
                )
            gs16 = cst.tile((SHF, S), bf16, name="gs16", tag="gs16")
            if not sim_gelu:
                nc.scalar.activation(gs16[:], su_ps[:], Act.Gelu_apprx_tanh)
            else:
                su16 = tmp.tile((SHF, S), bf16, name="su16", tag="u16", bufs=2)
                nc.vector.tensor_copy(su16[:], su_ps[:])
                sx2 = tmp.tile((SHF, S), bf16, name="sx2", tag="x2", bufs=2)
                nc.vector.tensor_tensor(sx2[:], su16[:], su16[:], Alu.mult)
                st1 = tmp.tile((SHF, S), bf16, name="st1", tag="x2", bufs=2)
                nc.vector.tensor_scalar(st1[:], sx2[:], GELU_A, 1.0,
                                        op0=Alu.mult, op1=Alu.add)
                smm = tmp.tile((SHF, S), bf16, name="smm", tag="x2", bufs=2)
                nc.vector.tensor_tensor(smm[:], su16[:], st1[:], Alu.mult)
                ssg = tmp.tile((SHF, S), bf16, name="ssg", tag="x2", bufs=2)
                nc.scalar.activation(ssg[:], smm[:], Act.Sigmoid, scale=GELU_C)
                nc.vector.tensor_tensor(gs16[:], su16[:], ssg[:], Alu.mult)

            # ---------- router (bf16 matmul, fp32 top-2 on logits) ----------
            lg_ps = psp.tile((P, TCH, E), f32, name="lg_ps", tag="lg", bufs=1)
            for t in range(TCH):
                for c in range(DC):
                    nc.tensor.matmul(
                        lg_ps[:, t, :], lhsT=h2T16[c][:, ts(t, P)],
                        rhs=gsl("gw", c, E),
                        start=(c == 0), stop=(c == DC - 1),
                    )
            gb_ap = psl("gb").rearrange("p (t e) -> p t e", e=E)
            lg32 = cst.tile((P, TCH, E), f32, name="lg32", tag="lg32")
            nc.vector.tensor_tensor(lg32[:], lg_ps[:], gb_ap, Alu.add)
            ex32 = cst.tile((P, TCH, E), f32, name="ex32", tag="ex32")
            nc.scalar.activation(ex32[:], lg32[:], Act.Exp, scale=1.0)
            ssum4 = cst.tile((P, TCH), f32, name="ssum4", tag="ssum4")
            nc.vector.reduce_sum(ssum4[:], ex32[:], axis=AX.X)
            rinv4 = cst.tile((P, TCH), f32, name="rinv4", tag="rinv4")
            nc.vector.reciprocal(rinv4[:], ssum4[:])
            m1 = cst.tile((P, TCH), f32, name="m1", tag="m1")
            nc.vector.reduce_max(m1[:], lg32[:], axis=AX.X)
            ge1 = cst.tile((P, TCH, E), f32, name="ge1", tag="ge1")
            nc.vector.tensor_tensor(
                ge1[:], lg32[:], m1[:, :, None].to_broadcast((P, TCH, E)),
                Alu.is_ge,
            )
            msk = cst.tile((P, TCH, E), f32, name="msk", tag="msk")
            nc.vector.scalar_tensor_tensor(
                msk[:], ge1[:], -1e30, lg32[:], op0=Alu.mult, op1=Alu.add
            )
            m2 = cst.tile((P, TCH), f32, name="m2", tag="m2")
            nc.vector.reduce_max(m2[:], msk[:], axis=AX.X)
            ge2 = cst.tile((P, TCH, E), f32, name="ge2", tag="ge2")
            nc.vector.tensor_tensor(
                ge2[:], lg32[:], m2[:, :, None].to_broadcast((P, TCH, E)),
                Alu.is_ge,
            )
            wgt = cst.tile((P, TCH, E), f32, name="wgt", tag="wgt")
            nc.vector.tensor_tensor(wgt[:], ex32[:], ge2[:], Alu.mult)
            wg = cst.tile((P, TCH, E), f32, name="wg", tag="wg")
            nc.vector.tensor_tensor(
                wg[:], wgt[:], rinv4[:, :, None].to_broadcast((P, TCH, E)),
                Alu.mult,
            )

            # transpose the two local experts' gate columns, broadcast across
            # partitions with a rank-1 matmul, scale by 16 (fp8 headroom).
            # rp_ps stays in PSUM; the gate-mults read it directly.
            wrow = [
                cst.tile((1, S), bf16, name=f"wrow{el}", tag=f"wrow{el}")
                for el in range(2)
            ]
            for t in range(TCH):
                for el in range(2):
                    wt_ps = psp.tile((1, P), f32, name="wt_ps",
                                     tag=("lg" if el else "avT"), bufs=1)
                    nc.tensor.transpose(wt_ps[:], wg[:, t, el:el + 1], idf)
                    nc.vector.tensor_scalar(wrow[el][:, ts(t, P)], wt_ps[:],
                                            float(WS), None, op0=Alu.mult)
            rp_ps = []
            for el in range(2):
                rp = psp.tile((P, S), f32, name=f"rp_ps{el}",
                              tag=("lg" if el else "avT"), bufs=1)
                nc.tensor.matmul(
                    rp[:], lhsT=ones[0:1, :], rhs=wrow[el][:],
                    start=True, stop=True,
                )
                rp_ps.append(rp)

            # gate the gelu outputs into fp8 DoubleRow pair tiles
            g0f8 = [
                [
                    cst.tile((P, 2, S), f8, name=f"g0f8_{el}_{p}",
                             tag=f"g0f8_{el}_{p}")
                    for p in range(2)
                ]
                for el in range(2)
            ]
            for el in range(2):
                for fc in range(FCH):
                    nc.vector.tensor_tensor(
                        g0f8[el][fc // 2][:, fc % 2, :], g0b[el][fc][:],
                        rp_ps[el][:], Alu.mult,
                    )

            # ---------- down-projections (fp8 DoubleRow) ----------
            rs_in = drp.tile((Dm, S), f8, name="rs_in")
            for m in range(DC):
                moe_ps = psp.tile((P, S), f32, name=f"moe_ps{m}", tag="moe",
                                  bufs=4)
                first = True
                for el in range(2):
                    for p in range(2):
                        nc.tensor.matmul(
                            moe_ps[:], lhsT=ev8(m, el, p), rhs=g0f8[el][p][:],
                            start=first, stop=False, perf_mode=DR,
                        )
                        first = False
                nc.tensor.matmul(
                    moe_ps[:], lhsT=gsl("sv", m, P)[0:SHF, :], rhs=gs16[:],
                    start=False, stop=True,
                )
                fin = tmp.tile((P, S), f8, name="fin", tag="fin", bufs=2)
                nc.vector.tensor_copy(fin[:], moe_ps[:])
                nc.sync.dma_start(rs_in[ts(m, P), :], fin[:])

            # ---------- fp8 ReduceScatter of MoE + exact local residual ----
            rs_out = drp.tile((DSH, S), f8, name="rs_out")
            nc.gpsimd.collective_compute(
                "ReduceScatter", Alu.add,
                replica_groups=[list(range(NCORES))],
                ins=[rs_in[:]], outs=[rs_out[:]],
            )
            rs_sb = cst.tile((DSH, S), f8, name="rs_sb", tag="rs_sb")
            nc.sync.dma_start(rs_sb[:], rs_out[:])
            out32 = cst.tile((DSH, S), f32, name="out32", tag="out32")
            nc.vector.scalar_tensor_tensor(
                out32[:], rs_sb[:], float(OSC), xsel[:],
                op0=Alu.mult, op1=Alu.add,
            )
            nc.sync.dma_start(outT_d[:], out32[:])

    nc.compile()
    return nc


def _prep_inputs(inputs):
    """Host-side sharding/layout prep. Returns (in_maps, a1, a2, full_mask)."""
    x = np.asarray(inputs["x"], np.float32)            # [1,S,D]
    attn_mask = np.asarray(inputs["attn_mask"])        # [1,S]
    wq = np.asarray(inputs["wq"], np.float32)
    wk = np.asarray(inputs["wk"], np.float32)
    wv = np.asarray(inputs["wv"], np.float32)
    wo = np.asarray(inputs["wo"], np.float32)
    a1 = float(np.asarray(inputs["a1"]).reshape(-1)[0])
    g1 = np.asarray(inputs["g1"], np.float32).reshape(Dm)
    b1 = np.asarray(inputs["b1"], np.float32).reshape(Dm)
    a2 = float(np.asarray(inputs["a2"]).reshape(-1)[0])
    g2 = np.asarray(inputs["g2"], np.float32).reshape(Dm)
    b2 = np.asarray(inputs["b2"], np.float32).reshape(Dm)
    gate_w = np.asarray(inputs["gate_w"], np.float32)  # [D,E]
    gate_b = np.asarray(inputs["gate_b"], np.float32).reshape(E)
    ek = np.asarray(inputs["ek"], np.float32)          # [E,D,FF]
    ev = np.asarray(inputs["ev"], np.float32)          # [E,FF,D]
    sk = np.asarray(inputs["sk"], np.float32)          # [1,D,FF]
    sv = np.asarray(inputs["sv"], np.float32)          # [1,FF,D]

    xT = np.ascontiguousarray(x[0].T)                  # [D,S]
    # chunk-major pack: [128, 4*512]
    xTp = np.concatenate([xT[i * P:(i + 1) * P, :] for i in range(DC)], axis=1)

    # rope tables (transposed layout: [freq, pos])
    pos = np.arange(S, dtype=np.float32)
    half = HD // 2
    inv = 1.0 / (10000.0 ** (np.arange(half, dtype=np.float32) / half))
    ang = pos[:, None] * inv[None, :]                  # [S, half]
    cosT = np.cos(ang).T.astype(np.float32)            # [32,S]
    sinT = np.sin(ang).T.astype(np.float32)
    cd = np.concatenate([cosT, cosT, cosT, cosT], 0)
    cs = np.concatenate([-sinT, sinT, -sinT, sinT], 0)

    # additive attention mask, exactly as the reference builds it, but stored
    # TRANSPOSED ([k, q]) for the scoresT layout.
    causal = np.tril(np.ones((S, S), np.float32))
    am = attn_mask.astype(np.float32)[0]               # [S]
    cm = causal * am[None, :]
    cm[np.arange(S), np.arange(S)] = 1.0
    addmask = -(1.0 - cm) * 1e9                        # [S,S] ([q, k])
    addmaskT = np.ascontiguousarray(addmask.T)         # [k, q]
    offdiag_needed = any(
        np.any(addmask[i * P:(i + 1) * P, : i * P] != 0.0)
        for i in range(1, TCH)
    )
    full_mask = bool(offdiag_needed)
    if full_mask:
        # block j: [128 k-rows, S q-cols], valid region [:, :S-128j]
        mblocks = []
        for j in range(TCH):
            blk = np.zeros((P, S), np.float32)
            blk[:, :S - P * j] = addmaskT[j * P:(j + 1) * P, P * j:]
            mblocks.append(blk)
    else:
        mblocks = [addmaskT[i * P:(i + 1) * P, i * P:(i + 1) * P]
                   for i in range(TCH)]

    (atn_l, atn_c), (mid_l, mid_c), (moe_l, moe_c), (p32_l, p32_c) = \
        _layouts(full_mask)

    def pack(layout, total, blocks, dtype):
        arr = np.zeros((P, total), dtype)
        for name, data in blocks.items():
            off, cols = layout[name]
            data = np.asarray(data, np.float32)
            assert data.shape[1] == cols, (name, data.shape, cols)
            arr[:data.shape[0], off:off + cols] = data.astype(dtype)
        return arr

    def cat(chunks):
        return np.concatenate(chunks, axis=1)

    wo_pk = cat([wo[i * P:(i + 1) * P, :] for i in range(DC)])
    id128 = np.eye(P, dtype=np.float32)

    # sel4[k, i*64+m] = (k == i): rank-1 selector rows for the bcast matmuls
    sel4 = np.zeros((TCH, TCH * HD), np.float32)
    for i in range(TCH):
        sel4[i, i * HD:(i + 1) * HD] = 1.0

    common32 = {
        "g1": np.stack([g1[i * P:(i + 1) * P] for i in range(DC)], 1),
        "b1": np.stack([b1[i * P:(i + 1) * P] for i in range(DC)], 1),
        "g2": np.stack([g2[i * P:(i + 1) * P] for i in range(DC)], 1),
        "b2": np.stack([b2[i * P:(i + 1) * P] for i in range(DC)], 1),
        "idf": id128,
    }

    warm = np.eye(P, dtype=BF)

    in_maps = []
    for c in range(NCORES):
        hsl = slice(c * HD, (c + 1) * HD)
        wqk = np.concatenate([wq[:, hsl] * 0.125, wk[:, hsl]], axis=1)
        wqk_pk = cat([wqk[i * P:(i + 1) * P, :] for i in range(DC)])
        wv_pk = cat([wv[i * P:(i + 1) * P, hsl] for i in range(DC)])
        atn_pack = pack(atn_l, atn_c, {
            "wqk": wqk_pk, "wv": wv_pk, "cd": cd, "cs": cs,
            "mask": cat(mblocks), "idbf": id128,
            "ones": np.ones((P, P), np.float32), "sel4": sel4,
        }, BF)

        perm = [2 * c, 2 * c + 1] + [e for e in range(E)
                                     if e not in (2 * c, 2 * c + 1)]
        gwp = gate_w[:, perm]
        gbp = gate_b[perm]
        rsl = slice(c * DSH, (c + 1) * DSH)
        mid_pack = pack(mid_l, mid_c, {
            "wo": wo_pk,
            "wosel": cat([wo[i * P:(i + 1) * P, rsl] for i in range(DC)]),
            "gw": cat([gwp[i * P:(i + 1) * P, :] for i in range(DC)]),
            "sk": cat([sk[0][i * P:(i + 1) * P, c * SHF:(c + 1) * SHF]
                       for i in range(DC)]),
            "sv": sv[0][c * SHF:(c + 1) * SHF, :] * (WS * WS),
        }, BF)

        p32_pack = pack(p32_l, p32_c, dict(
            common32,
            gb=np.tile(gbp, (P, TCH)),
            xrows=xT[rsl, :],
        ), np.float32)

        # fp8 DoubleRow packs (pre-scaled by WS)
        ekb = []
        for el in range(2):
            eke = ek[2 * c + el] * WS                  # [D, FF]
            for fc in range(FCH):
                for p in range(2):
                    for k in range(2):
                        d0 = (2 * p + k) * P
                        ekb.append(eke[d0:d0 + P, fc * P:(fc + 1) * P])
        evb = []
        for m in range(DC):
            for el in range(2):
                eve = ev[2 * c + el] * WS              # [FF, D]
                for p in range(2):
                    for k in range(2):
                        f0 = (2 * p + k) * P
                        evb.append(eve[f0:f0 + P, m * P:(m + 1) * P])
        moe_pack = pack(moe_l, moe_c, {
            "ek": cat(ekb), "ev": cat(evb),
        }, F8)

        in_maps.append(dict(
            warm16=warm, xT16=xTp.astype(BF),
            atn16=atn_pack, mid16=mid_pack, moe8=moe_pack, p32=p32_pack,
        ))
    return in_maps, a1, a2, full_mask


def kernel(**inputs):
    from concourse import bass_utils

    sim = bool(os.environ.get("BASSK_SIM"))
    sim_gelu = sim or bool(os.environ.get("BASSK_COMPOSED_GELU"))
    in_maps, a1v, a2v, full_mask = _prep_inputs(inputs)
    key = (a1v, a2v, full_mask, sim_gelu)
    if key not in _PROG_CACHE:
        _PROG_CACHE[key] = _build_program(a1v, a2v, full_mask, sim_gelu)
    nc = _PROG_CACHE[key]

    if sim:
        from concourse.bass_interp import MultiCoreSim

        simu = MultiCoreSim(nc, num_cores=NCORES)
        for c in range(NCORES):
            for k, v in in_maps[c].items():
                simu.cores[c].tensor(k)[:] = v
        simu.simulate(check_with_hw=False)
        shards = [np.array(simu.cores[c].tensor("outT")) for c in range(NCORES)]
    else:
        trace = bool(os.environ.get("BASSK_TRACE"))
        res = bass_utils.run_bass_kernel_spmd(
            nc, in_maps, core_ids=list(range(NCORES)), trace=trace
        )
        LAST_INFO["exec_time_ns"] = res.exec_time_ns
        LAST_INFO["profile_json"] = res.profile_json
        shards = [np.asarray(res.results[c]["outT"]) for c in range(NCORES)]

    outT = np.concatenate(shards, axis=0)              # [D,S]
    return np.ascontiguousarray(outT.T).reshape(1, S, Dm).astype(np.float32)
